# revision 1
# baseline (speedup 1.0000x reference)
"""Bass/Trainium2 kernel for nn_KernelizedAttentionResBlock.

Sharding: n-token sharded phases (each of 8 cores owns a 128-row slice of
n for ALL batches -> K/V slices, mu_w/sigma_w column slices), one small
AllGather of x, then m-sharded FFN (each core owns 512 of the 4096 hidden
units for all batches).  Host pre-transposes weights (so no on-chip weight
transposes are needed), and the host sums the 8 partial FFN outputs.

All LayerNorm gains/biases are folded exactly into the following linear
layers on the host, so the device only ever normalizes.
"""
import sys
import os

sys.path.insert(0, "/opt/trn_rl_repo")

import numpy as np

N = 1024          # n_token
B = 32            # batch
D = 1024          # broadcast dim of K/V
M = 4096          # FFN hidden
NCORES = 8
NSL = N // NCORES     # 128 rows of n per core
MSL = M // NCORES     # 512 FFN hidden units per core
MCH = MSL // 128      # 4 chunks of 128
LN_EPS = 1e-5
F32 = None  # set after mybir import

_built = {}
last_results = None  # BassKernelResults of the most recent run (for profiling)


def _build_module():
    """Build (once) the SPMD Bass module run on every core."""
    if "nc" in _built:
        return _built["nc"]

    import concourse.bacc as bacc
    import concourse.mybir as mybir
    import concourse.tile as tile

    AF = mybir.ActivationFunctionType
    ALU = mybir.AluOpType
    AX = mybir.AxisListType
    f32 = mybir.dt.float32

    nc = bacc.Bacc(trn_type="TRN2", num_devices=NCORES)

    Kd = nc.dram_tensor("Ks", (NSL, B, D), f32, kind="ExternalInput")
    Vd = nc.dram_tensor("Vs", (NSL, B, D), f32, kind="ExternalInput")
    Qf = nc.dram_tensor("Qf", (B, N), f32, kind="ExternalInput")
    QTs = nc.dram_tensor("QTs", (NSL, B), f32, kind="ExternalInput")
    MWT = nc.dram_tensor("MWT", (N, NSL), f32, kind="ExternalInput")
    SWT = nc.dram_tensor("SWT", (N, NSL), f32, kind="ExternalInput")
    MB2 = nc.dram_tensor("MB2", (NSL, 1), f32, kind="ExternalInput")
    SB1 = nc.dram_tensor("SB1", (NSL, 1), f32, kind="ExternalInput")
    W1T = nc.dram_tensor("W1T", (N, MSL), f32, kind="ExternalInput")
    B1P = nc.dram_tensor("B1P", (128, MCH), f32, kind="ExternalInput")
    B1N = nc.dram_tensor("B1N", (128, MCH), f32, kind="ExternalInput")
    W2T = nc.dram_tensor("W2T", (MSL, N), f32, kind="ExternalInput")
    IDT = nc.dram_tensor("IDT", (128, 128), f32, kind="ExternalInput")

    XTd = nc.dram_tensor("XT", (NSL, B), f32, kind="ExternalOutput")
    HPd = nc.dram_tensor("HP", (N, B), f32, kind="ExternalOutput")

    cc_in = nc.dram_tensor("cc_in", (B, NSL), f32, kind="Internal")
    cc_out = nc.dram_tensor(
        "cc_out", (NCORES * B, NSL), f32, kind="Internal", addr_space="Shared"
    )

    with tile.TileContext(nc) as tc:
        with tc.tile_pool(name="const", bufs=1) as cst, \
             tc.tile_pool(name="small", bufs=1) as sm, \
             tc.tile_pool(name="kv", bufs=4) as kv, \
             tc.tile_pool(name="scr", bufs=3) as scr, \
             tc.tile_pool(name="psum", bufs=1, space="PSUM") as ps:

            ident = cst.tile([128, 128], f32)
            nc.sync.dma_start(ident[:], IDT[:])

            # ---------- Phase 0: q = LayerNorm(Q) on [32, 1024] ----------
            qf = sm.tile([B, N], f32)
            nc.sync.dma_start(qf[:], Qf[:])
            qsum = sm.tile([B, 1], f32)
            nc.vector.reduce_sum(qsum[:], qf[:], axis=AX.X)
            negmean = sm.tile([B, 1], f32)
            nc.vector.tensor_scalar_mul(negmean[:], qsum[:], -1.0 / N)
            tq = sm.tile([B, N], f32)
            nc.scalar.activation(tq[:], qf[:], AF.Identity, bias=negmean[:])
            sqq = sm.tile([B, N], f32)
            nc.scalar.activation(sqq[:], qf[:], AF.Square, bias=negmean[:])
            ss = sm.tile([B, 1], f32)
            nc.vector.reduce_sum(ss[:], sqq[:], axis=AX.X)
            vv = sm.tile([B, 1], f32)
            nc.vector.tensor_scalar(vv[:], ss[:], 1.0 / N, LN_EPS,
                                    op0=ALU.mult, op1=ALU.add)
            lv = sm.tile([B, 1], f32)
            nc.scalar.activation(lv[:], vv[:], AF.Ln)
            rstd = sm.tile([B, 1], f32)
            nc.scalar.activation(rstd[:], lv[:], AF.Exp, scale=-0.5)
            qn = sm.tile([B, N], f32)
            nc.vector.tensor_scalar_mul(qn[:], tq[:], rstd[:])

            # qT chunks: [32, 128] -> [128, 32] PE transposes
            qt_sb = cst.tile([128, NCORES, B], f32)
            for c in range(NCORES):
                tp = ps.tile([128, B], f32, tag="tp")
                nc.tensor.transpose(tp[:], qn[:, c * 128:(c + 1) * 128],
                                    ident[:B, :B])
                nc.scalar.copy(qt_sb[:, c, :], tp[:])

            # ---------- Phase 1: mu / sigma for our n-slice ----------
            muwT = cst.tile([128, NCORES, NSL], f32)
            nc.sync.dma_start(muwT[:], MWT[:].rearrange("(c p) j -> p c j", p=128))
            sigwT = cst.tile([128, NCORES, NSL], f32)
            nc.sync.dma_start(sigwT[:], SWT[:].rearrange("(c p) j -> p c j", p=128))
            mb2 = cst.tile([NSL, 1], f32)
            nc.sync.dma_start(mb2[:], MB2[:])
            sb1 = cst.tile([NSL, 1], f32)
            nc.sync.dma_start(sb1[:], SB1[:])

            mu_ps = ps.tile([NSL, B], f32, tag="mmu")
            for c in range(NCORES):
                nc.tensor.matmul(mu_ps[:], muwT[:, c, :], qt_sb[:, c, :],
                                 start=(c == 0), stop=(c == NCORES - 1))
            # -tanh(z) = 2/(exp(2z)+1) - 1 ;  exp(2z) = Exp(2*psum + 2*mu_b)
            e2 = sm.tile([NSL, B], f32)
            nc.scalar.activation(e2[:], mu_ps[:], AF.Exp, scale=2.0, bias=mb2[:])
            d1 = sm.tile([NSL, B], f32)
            nc.vector.tensor_scalar_add(d1[:], e2[:], 1.0)
            r1 = sm.tile([NSL, B], f32)
            nc.vector.reciprocal(r1[:], d1[:])
            negmu = sm.tile([NSL, B], f32)
            nc.vector.tensor_scalar(negmu[:], r1[:], 2.0, -1.0,
                                    op0=ALU.mult, op1=ALU.add)

            sig_ps = ps.tile([NSL, B], f32, tag="msig")
            for c in range(NCORES):
                nc.tensor.matmul(sig_ps[:], sigwT[:, c, :], qt_sb[:, c, :],
                                 start=(c == 0), stop=(c == NCORES - 1))
            s2 = sm.tile([NSL, B], f32)
            nc.scalar.activation(s2[:], sig_ps[:], AF.Square, bias=sb1[:])
            s2e = sm.tile([NSL, B], f32)
            nc.vector.tensor_scalar_add(s2e[:], s2[:], 1e-8)
            rs = sm.tile([NSL, B], f32)
            nc.vector.reciprocal(rs[:], s2e[:])
            cT = sm.tile([NSL, B], f32)
            nc.vector.tensor_scalar_mul(cT[:], rs[:], -0.5)

            # ---------- Phase 2: stream K/V, A = sum_D exp(...)*V ----------
            xT = sm.tile([NSL, B], f32)
            NB = 2  # batches per DMA block (1MB transfers)
            for blk in range(B // NB):
                b0 = blk * NB
                kt = kv.tile([NSL, NB, D], f32, tag="kt")
                nc.sync.dma_start(kt[:], Kd[:, b0:b0 + NB, :])
                vt = kv.tile([NSL, NB, D], f32, tag="vt")
                nc.sync.dma_start(vt[:], Vd[:, b0:b0 + NB, :])
                for bi in range(NB):
                    b = b0 + bi
                    sq = scr.tile([NSL, D], f32, tag="sq")
                    nc.scalar.activation(sq[:], kt[:, bi, :], AF.Square,
                                         bias=negmu[:, b:b + 1])
                    es = scr.tile([NSL, D], f32, tag="es")
                    nc.scalar.activation(es[:], sq[:], AF.Exp,
                                         scale=cT[:, b:b + 1])
                    sv = scr.tile([NSL, D], f32, tag="sv")
                    nc.vector.tensor_mul(sv[:], es[:], vt[:, bi, :])
                    nc.vector.reduce_sum(xT[:, b:b + 1], sv[:], axis=AX.X)

            qts = cst.tile([NSL, B], f32)
            nc.sync.dma_start(qts[:], QTs[:])
            xT2 = sm.tile([NSL, B], f32)
            nc.vector.tensor_add(xT2[:], xT[:], qts[:])
            nc.sync.dma_start(XTd[:], xT2[:])

            # ---------- Phase 3: AllGather x, LN, m-sharded FFN ----------
            xnat_ps = ps.tile([B, NSL], f32, tag="tx")
            nc.tensor.transpose(xnat_ps[:], xT2[:], ident[:])
            xnat = sm.tile([B, NSL], f32)
            nc.scalar.copy(xnat[:], xnat_ps[:])
            nc.sync.dma_start(cc_in[:], xnat[:])
            nc.gpsimd.collective_compute(
                "AllGather", ALU.bypass,
                replica_groups=[list(range(NCORES))],
                ins=[cc_in[:]], outs=[cc_out[:]],
            )
            xf = sm.tile([B, N], f32)
            nc.sync.dma_start(
                xf[:].rearrange("b (c j) -> b c j", c=NCORES),
                cc_out[:].rearrange("(c b) j -> b c j", b=B),
            )
            # LayerNorm(x)
            xsum = sm.tile([B, 1], f32)
            nc.vector.reduce_sum(xsum[:], xf[:], axis=AX.X)
            xnegmean = sm.tile([B, 1], f32)
            nc.vector.tensor_scalar_mul(xnegmean[:], xsum[:], -1.0 / N)
            tx = sm.tile([B, N], f32)
            nc.scalar.activation(tx[:], xf[:], AF.Identity, bias=xnegmean[:])
            sqx = sm.tile([B, N], f32)
            nc.scalar.activation(sqx[:], xf[:], AF.Square, bias=xnegmean[:])
            ssx = sm.tile([B, 1], f32)
            nc.vector.reduce_sum(ssx[:], sqx[:], axis=AX.X)
            vvx = sm.tile([B, 1], f32)
            nc.vector.tensor_scalar(vvx[:], ssx[:], 1.0 / N, LN_EPS,
                                    op0=ALU.mult, op1=ALU.add)
            lvx = sm.tile([B, 1], f32)
            nc.scalar.activation(lvx[:], vvx[:], AF.Ln)
            rstdx = sm.tile([B, 1], f32)
            nc.scalar.activation(rstdx[:], lvx[:], AF.Exp, scale=-0.5)
            hn = sm.tile([B, N], f32)
            nc.vector.tensor_scalar_mul(hn[:], tx[:], rstdx[:])

            ht_sb = cst.tile([128, NCORES, B], f32)
            for c in range(NCORES):
                tp2 = ps.tile([128, B], f32, tag="tp")
                nc.tensor.transpose(tp2[:], hn[:, c * 128:(c + 1) * 128],
                                    ident[:B, :B])
                nc.scalar.copy(ht_sb[:, c, :], tp2[:])

            w1T = cst.tile([128, NCORES, MSL], f32)
            nc.sync.dma_start(w1T[:], W1T[:].rearrange("(c p) m -> p c m", p=128))
            b1p = cst.tile([128, MCH], f32)
            nc.sync.dma_start(b1p[:], B1P[:])
            b1n = cst.tile([128, MCH], f32)
            nc.sync.dma_start(b1n[:], B1N[:])

            g1_sb = sm.tile([128, MCH, B], f32)
            for mi in range(MCH):
                h1_ps = ps.tile([128, B], f32, tag="h1")
                for c in range(NCORES):
                    nc.tensor.matmul(h1_ps[:],
                                     w1T[:, c, mi * 128:(mi + 1) * 128],
                                     ht_sb[:, c, :],
                                     start=(c == 0), stop=(c == NCORES - 1))
                # silu(z) = z / (1 + exp(-z)), z = psum + b1
                z = sm.tile([128, B], f32, tag="z")
                nc.scalar.activation(z[:], h1_ps[:], AF.Identity,
                                     bias=b1p[:, mi:mi + 1])
                em = sm.tile([128, B], f32, tag="em")
                nc.scalar.activation(em[:], h1_ps[:], AF.Exp, scale=-1.0,
                                     bias=b1n[:, mi:mi + 1])
                dd = sm.tile([128, B], f32, tag="dd")
                nc.vector.tensor_scalar_add(dd[:], em[:], 1.0)
                rr = sm.tile([128, B], f32, tag="rr")
                nc.vector.reciprocal(rr[:], dd[:])
                nc.vector.tensor_mul(g1_sb[:, mi, :], z[:], rr[:])

            w2T = cst.tile([128, MCH, N], f32)
            nc.sync.dma_start(w2T[:], W2T[:].rearrange("(mi p) n -> p mi n", p=128))
            hp_sb = sm.tile([128, NCORES, B], f32)
            for jn in range(NCORES):
                hp_ps = ps.tile([128, B], f32, tag="hp")
                for mi in range(MCH):
                    nc.tensor.matmul(hp_ps[:],
                                     w2T[:, mi, jn * 128:(jn + 1) * 128],
                                     g1_sb[:, mi, :],
                                     start=(mi == 0), stop=(mi == MCH - 1))
                nc.scalar.copy(hp_sb[:, jn, :], hp_ps[:])
            nc.sync.dma_start(
                HPd[:].rearrange("(jn p) b -> p jn b", p=128), hp_sb[:]
            )

    nc.finalize()
    _built["nc"] = nc
    return nc


def kernel(**inputs):
    from concourse.bass_utils import run_bass_kernel_spmd

    global last_results

    Q = np.asarray(inputs["Q"], dtype=np.float32)
    K = np.asarray(inputs["K"], dtype=np.float32)
    V = np.asarray(inputs["V"], dtype=np.float32)
    mu_w = np.asarray(inputs["mu_w"], dtype=np.float32)
    mu_b = np.asarray(inputs["mu_b"], dtype=np.float32)
    sigma_w = np.asarray(inputs["sigma_w"], dtype=np.float32)
    sigma_b = np.asarray(inputs["sigma_b"], dtype=np.float32)
    ffn_w1 = np.asarray(inputs["ffn_w1"], dtype=np.float32)
    ffn_b1 = np.asarray(inputs["ffn_b1"], dtype=np.float32)
    ffn_w2 = np.asarray(inputs["ffn_w2"], dtype=np.float32)
    ffn_b2 = np.asarray(inputs["ffn_b2"], dtype=np.float32)
    ln_ff_g = np.asarray(inputs["ln_ff_g"], dtype=np.float32)
    ln_ff_b = np.asarray(inputs["ln_ff_b"], dtype=np.float32)
    ln_q_g = np.asarray(inputs["ln_q_g"], dtype=np.float32)
    ln_q_b = np.asarray(inputs["ln_q_b"], dtype=np.float32)

    # ---- Host-side exact folds of LN affine params into next matmuls ----
    # q = t*g + b  =>  q @ W.T + c = t @ (W*g).T + (c + W @ b)
    mu_wf = mu_w * ln_q_g[None, :]
    mu_bf = mu_b + mu_w @ ln_q_b
    sig_wf = sigma_w * ln_q_g[None, :]
    sig_bf = sigma_b + sigma_w @ ln_q_b
    w1f = ffn_w1 * ln_ff_g[None, :]
    b1f = ffn_b1 + ffn_w1 @ ln_ff_b

    QT = np.ascontiguousarray(Q.T)                    # (N, B)
    muwT = np.ascontiguousarray(mu_wf.T)              # (N, N)  [jn, j]
    sigwT = np.ascontiguousarray(sig_wf.T)
    w1T = np.ascontiguousarray(w1f.T)                 # (N, M)
    w2T = np.ascontiguousarray(ffn_w2.T)              # (M, N)
    ident = np.eye(128, dtype=np.float32)

    nc = _build_module()

    in_maps = []
    for c in range(NCORES):
        jsl = slice(c * NSL, (c + 1) * NSL)
        msl = slice(c * MSL, (c + 1) * MSL)
        b1s = b1f[msl]
        in_maps.append({
            "Ks": np.ascontiguousarray(K[:, jsl, :].transpose(1, 0, 2)),
            "Vs": np.ascontiguousarray(V[:, jsl, :].transpose(1, 0, 2)),
            "Qf": Q,
            "QTs": np.ascontiguousarray(QT[jsl, :]),
            "MWT": np.ascontiguousarray(muwT[:, jsl]),
            "SWT": np.ascontiguousarray(sigwT[:, jsl]),
            "MB2": np.ascontiguousarray(2.0 * mu_bf[jsl]).reshape(NSL, 1),
            "SB1": np.ascontiguousarray(sig_bf[jsl]).reshape(NSL, 1),
            "W1T": np.ascontiguousarray(w1T[:, msl]),
            "B1P": np.ascontiguousarray(b1s.reshape(MCH, 128).T),
            "B1N": np.ascontiguousarray((-b1s).reshape(MCH, 128).T),
            "W2T": np.ascontiguousarray(w2T[msl, :]),
            "IDT": ident,
        })

    trace = os.environ.get("BASS_KERNEL_TRACE", "0") == "1"
    res = run_bass_kernel_spmd(
        nc, in_maps, core_ids=list(range(NCORES)), trace=trace
    )
    last_results = res

    x = np.concatenate([res.results[c]["XT"] for c in range(NCORES)], axis=0).T
    h = np.zeros((N, B), dtype=np.float32)
    for c in range(NCORES):
        h += res.results[c]["HP"]
    out = x + h.T + ffn_b2[None, :]
    return out.astype(np.float32)



# revision 9
# speedup vs baseline: 1.3399x; 1.3399x over previous
"""Bass/Trainium2 kernel for nn_KernelizedAttentionResBlock (optimized v2).

Sharding: n-token sharded stream phase (each of 8 cores owns a 128-row
slice of n for ALL batches), one AllGather of x, m-sharded FFN.

Key optimizations over v1:
 - K/V streamed in fp16 (halves the DMA-bound stream phase).
 - Gaussian computed as exp(-(s*K+t)^2) with s=rsqrt(2*sigma^2+2e-8),
   t=-s*mu folded per (token,batch): one fused square op per batch
   (Act engine: Square with scale+bias, or DVE: tensor_scalar 4x +
   tensor_tensor 2x), one batch-agnostic big-tile Exp on Act.
 - S*V multiply + D-reduction + Q-residual in ONE DVE
   tensor_tensor_reduce (scalar arg = Q^T initializes the reduction).
 - All matmul weights in fp16 (1 cycle/row on PE vs 4 for fp32,
   half the weight DMA bytes).
 - tanh/sigmoid/rsqrt computed directly by the Act engine.
 - LayerNorm statistics via bn_stats/bn_aggr (one DVE pass).
"""
import sys
import os

sys.path.insert(0, "/opt/trn_rl_repo")

import numpy as np

N = 1024          # n_token
B = 32            # batch
D = 1024          # broadcast dim of K/V
M = 4096          # FFN hidden
NCORES = 8
NSL = N // NCORES     # 128 rows of n per core
MSL = M // NCORES     # 512 FFN hidden units per core
MCH = MSL // 128      # 4 chunks of 128
LN_EPS = 1e-5
NB = 4                # batches per K/V DMA block
NACT_PER8 = 5         # of every 8 batches, this many take the Act-square path

_built = {}
last_results = None  # BassKernelResults of the most recent run (for profiling)


def _build_module():
    """Build (once) the SPMD Bass module run on every core."""
    if "nc" in _built:
        return _built["nc"]

    import concourse.bacc as bacc
    import concourse.mybir as mybir
    import concourse.tile as tile

    AF = mybir.ActivationFunctionType
    ALU = mybir.AluOpType
    f32 = mybir.dt.float32
    f16 = mybir.dt.float16
    SQ2 = float(np.sqrt(2.0))

    nc = bacc.Bacc(trn_type="TRN2", num_devices=NCORES)

    Kd = nc.dram_tensor("Ks", (NSL, B, D), f16, kind="ExternalInput")
    Vd = nc.dram_tensor("Vs", (NSL, B, D), f16, kind="ExternalInput")
    Qf = nc.dram_tensor("Qf", (B, N), f32, kind="ExternalInput")
    QTs = nc.dram_tensor("QTs", (NSL, B), f32, kind="ExternalInput")
    MWT = nc.dram_tensor("MWT", (N, NSL), f16, kind="ExternalInput")
    SWT = nc.dram_tensor("SWT", (N, NSL), f16, kind="ExternalInput")
    MB = nc.dram_tensor("MB", (NSL, 1), f32, kind="ExternalInput")
    SB2 = nc.dram_tensor("SB2", (NSL, 1), f32, kind="ExternalInput")
    W1T = nc.dram_tensor("W1T", (N, MSL), f16, kind="ExternalInput")
    B1 = nc.dram_tensor("B1", (128, MCH), f32, kind="ExternalInput")
    W2T = nc.dram_tensor("W2T", (MSL, N), f16, kind="ExternalInput")
    IDT = nc.dram_tensor("IDT", (128, 128), f32, kind="ExternalInput")
    IDH = nc.dram_tensor("IDH", (128, 128), f16, kind="ExternalInput")

    XTd = nc.dram_tensor("XT", (NSL, B), f32, kind="ExternalOutput")
    HPd = nc.dram_tensor("HP", (N, B), f32, kind="ExternalOutput")

    cc_in = nc.dram_tensor("cc_in", (B, NSL), f32, kind="Internal")
    cc_out = nc.dram_tensor(
        "cc_out", (NCORES * B, NSL), f32, kind="Internal", addr_space="Shared"
    )

    with tile.TileContext(nc) as tc:
        with tc.tile_pool(name="const", bufs=1) as cst, \
             tc.tile_pool(name="small", bufs=1) as sm, \
             tc.tile_pool(name="kv", bufs=3) as kv, \
             tc.tile_pool(name="sq", bufs=2) as sqp, \
             tc.tile_pool(name="es", bufs=2) as esp, \
             tc.tile_pool(name="scr", bufs=2) as scr, \
             tc.tile_pool(name="psum", bufs=1, space="PSUM") as ps:

            ident = cst.tile([128, 128], f32)
            nc.sync.dma_start(ident[:], IDT[:])
            identh = cst.tile([128, 128], f16)
            nc.sync.dma_start(identh[:], IDH[:])

            # Early weight/small loads (before K/V stream DMAs).
            qf = sm.tile([B, N], f32)
            nc.sync.dma_start(qf[:], Qf[:])
            muwT = cst.tile([128, NCORES, NSL], f16)
            nc.sync.dma_start(muwT[:], MWT[:].rearrange("(c p) j -> p c j", p=128))
            sigwT = cst.tile([128, NCORES, NSL], f16)
            nc.sync.dma_start(sigwT[:], SWT[:].rearrange("(c p) j -> p c j", p=128))
            mb = cst.tile([NSL, 1], f32)
            nc.sync.dma_start(mb[:], MB[:])
            sb2 = cst.tile([NSL, 1], f32)
            nc.sync.dma_start(sb2[:], SB2[:])
            qts = cst.tile([NSL, B], f32)
            nc.sync.dma_start(qts[:], QTs[:])

            # ---------- Phase 0: q = LayerNorm(Q) on [32, 1024] ----------
            qst = sm.tile([B, 2, 6], f32)
            nc.vector.bn_stats(qst[:, 0, :], qf[:, 0:512])
            nc.vector.bn_stats(qst[:, 1, :], qf[:, 512:1024])
            qmv = sm.tile([B, 2], f32)
            nc.vector.bn_aggr(qmv[:], qst[:])
            qve = sm.tile([B, 1], f32)
            nc.vector.tensor_scalar_add(qve[:], qmv[:, 1:2], LN_EPS)
            qvr = sm.tile([B, 1], f32)
            nc.vector.reciprocal(qvr[:], qve[:])
            rstd = sm.tile([B, 1], f32)
            nc.scalar.activation(rstd[:], qvr[:], AF.Sqrt)
            negmr = sm.tile([B, 1], f32)
            nc.vector.scalar_tensor_tensor(
                negmr[:], rstd[:], -1.0, qmv[:, 0:1], op0=ALU.mult, op1=ALU.mult)
            qn = sm.tile([B, N], f32)
            nc.vector.tensor_scalar(qn[:], qf[:], rstd[:], negmr[:],
                                    op0=ALU.mult, op1=ALU.add)

            # qT chunks: [32, 128] -> [128, 32] PE transposes, cast to fp16
            qt_sb = cst.tile([128, NCORES, B], f16)
            for c in range(NCORES):
                tp = ps.tile([128, B], f32, tag="tp")
                nc.tensor.transpose(tp[:], qn[:, c * 128:(c + 1) * 128],
                                    ident[:B, :B])
                nc.scalar.copy(qt_sb[:, c, :], tp[:])

            # ---------- Phase 1: per-(token,batch) scalars s, t ----------
            mu_ps = ps.tile([NSL, B], f32, tag="mmu")
            for c in range(NCORES):
                nc.tensor.matmul(mu_ps[:], muwT[:, c, :], qt_sb[:, c, :],
                                 start=(c == 0), stop=(c == NCORES - 1))
            mu = sm.tile([NSL, B], f32)
            nc.scalar.activation(mu[:], mu_ps[:], AF.Tanh, bias=mb[:])

            sig_ps = ps.tile([NSL, B], f32, tag="msig")
            for c in range(NCORES):
                nc.tensor.matmul(sig_ps[:], sigwT[:, c, :], qt_sb[:, c, :],
                                 start=(c == 0), stop=(c == NCORES - 1))
            # 2*sigma^2 = (sqrt2*sig + sqrt2*sig_b)^2
            s2 = sm.tile([NSL, B], f32)
            nc.scalar.activation(s2[:], sig_ps[:], AF.Square, scale=SQ2,
                                 bias=sb2[:])
            s2e = sm.tile([NSL, B], f32)
            nc.vector.tensor_scalar_add(s2e[:], s2[:], 2e-8)
            s2r = sm.tile([NSL, B], f32)
            nc.vector.reciprocal(s2r[:], s2e[:])
            sS = sm.tile([NSL, B], f32)
            nc.scalar.activation(sS[:], s2r[:], AF.Sqrt)
            tS = sm.tile([NSL, B], f32)
            nc.vector.scalar_tensor_tensor(
                tS[:], sS[:], -1.0, mu[:], op0=ALU.mult, op1=ALU.mult)

            # ---------- Phase 2: stream K/V ----------
            # x^T[j, b] = sum_d exp(-(s*K+t)^2) * V + Q^T[j, b]
            xT = sm.tile([NSL, B], f32)
            for blk in range(B // NB):
                b0 = blk * NB
                kt = kv.tile([NSL, NB, D], f16, tag="kt")
                nc.sync.dma_start(kt[:], Kd[:, b0:b0 + NB, :])
                vt = kv.tile([NSL, NB, D], f16, tag="vt")
                nc.sync.dma_start(vt[:], Vd[:, b0:b0 + NB, :])
                sq = sqp.tile([NSL, NB, D], f16, tag="sq")
                for i in range(NB):
                    b = b0 + i
                    if (b % 8) < NACT_PER8:
                        # Act path: (s*K + t)^2 in one fused op
                        nc.scalar.activation(sq[:, i, :], kt[:, i, :],
                                             AF.Square, scale=sS[:, b:b + 1],
                                             bias=tS[:, b:b + 1])
                    else:
                        # DVE path: tensor_scalar (4x) + self-mult (2x)
                        e = scr.tile([NSL, D], f16, tag="e")
                        nc.vector.tensor_scalar(e[:], kt[:, i, :],
                                                sS[:, b:b + 1], tS[:, b:b + 1],
                                                op0=ALU.mult, op1=ALU.add)
                        nc.vector.tensor_tensor(sq[:, i, :], e[:], e[:],
                                                op=ALU.mult)
                St = esp.tile([NSL, NB, D], f32, tag="st")
                nc.scalar.activation(
                    St[:].rearrange("p nb d -> p (nb d)"),
                    sq[:].rearrange("p nb d -> p (nb d)"),
                    AF.Exp, scale=-1.0)
                for i in range(NB):
                    b = b0 + i
                    sv = scr.tile([NSL, D], f32, tag="sv")
                    nc.vector.tensor_tensor_reduce(
                        sv[:], St[:, i, :], vt[:, i, :], 1.0, qts[:, b:b + 1],
                        op0=ALU.mult, op1=ALU.add, accum_out=xT[:, b:b + 1])

            nc.sync.dma_start(XTd[:], xT[:])

            # ---------- Phase 3: AllGather x ----------
            xnat_ps = ps.tile([B, NSL], f32, tag="tx")
            nc.tensor.transpose(xnat_ps[:], xT[:], ident[:])
            xnat = sm.tile([B, NSL], f32)
            nc.scalar.copy(xnat[:], xnat_ps[:])
            nc.sync.dma_start(cc_in[:], xnat[:])
            nc.gpsimd.collective_compute(
                "AllGather", ALU.bypass,
                replica_groups=[list(range(NCORES))],
                ins=[cc_in[:]], outs=[cc_out[:]],
            )
            xf = sm.tile([B, N], f32)
            nc.sync.dma_start(
                xf[:].rearrange("b (c j) -> b c j", c=NCORES),
                cc_out[:].rearrange("(c b) j -> b c j", b=B),
            )

            # FFN weights (issued after stream DMAs; needed only post-AG)
            w1T = cst.tile([128, NCORES, MSL], f16)
            nc.sync.dma_start(w1T[:], W1T[:].rearrange("(c p) m -> p c m", p=128))
            b1 = cst.tile([128, MCH], f32)
            nc.sync.dma_start(b1[:], B1[:])
            w2T = cst.tile([128, MCH, N], f16)
            nc.sync.dma_start(w2T[:], W2T[:].rearrange("(mi p) n -> p mi n", p=128))

            # ---------- Phase 4: h = LayerNorm(x), m-sharded FFN ----------
            xst = sm.tile([B, 2, 6], f32)
            nc.vector.bn_stats(xst[:, 0, :], xf[:, 0:512])
            nc.vector.bn_stats(xst[:, 1, :], xf[:, 512:1024])
            xmv = sm.tile([B, 2], f32)
            nc.vector.bn_aggr(xmv[:], xst[:])
            xve = sm.tile([B, 1], f32)
            nc.vector.tensor_scalar_add(xve[:], xmv[:, 1:2], LN_EPS)
            xvr = sm.tile([B, 1], f32)
            nc.vector.reciprocal(xvr[:], xve[:])
            rstdx = sm.tile([B, 1], f32)
            nc.scalar.activation(rstdx[:], xvr[:], AF.Sqrt)
            negmrx = sm.tile([B, 1], f32)
            nc.vector.scalar_tensor_tensor(
                negmrx[:], rstdx[:], -1.0, xmv[:, 0:1], op0=ALU.mult, op1=ALU.mult)
            hn = sm.tile([B, N], f32)
            nc.vector.tensor_scalar(hn[:], xf[:], rstdx[:], negmrx[:],
                                    op0=ALU.mult, op1=ALU.add)

            ht_sb = cst.tile([128, NCORES, B], f16)
            for c in range(NCORES):
                tp2 = ps.tile([128, B], f32, tag="tp")
                nc.tensor.transpose(tp2[:], hn[:, c * 128:(c + 1) * 128],
                                    ident[:B, :B])
                nc.scalar.copy(ht_sb[:, c, :], tp2[:])

            g1_sb = sm.tile([128, MCH, B], f16)
            for mi in range(MCH):
                h1_ps = ps.tile([128, B], f32, tag="h1")
                for c in range(NCORES):
                    nc.tensor.matmul(h1_ps[:],
                                     w1T[:, c, mi * 128:(mi + 1) * 128],
                                     ht_sb[:, c, :],
                                     start=(c == 0), stop=(c == NCORES - 1))
                # silu(z) = z * sigmoid(z), z = psum + b1
                sg = sm.tile([128, B], f32, tag="sg")
                nc.scalar.activation(sg[:], h1_ps[:], AF.Sigmoid,
                                     bias=b1[:, mi:mi + 1])
                nc.vector.scalar_tensor_tensor(
                    g1_sb[:, mi, :], h1_ps[:], b1[:, mi:mi + 1], sg[:],
                    op0=ALU.add, op1=ALU.mult)

            hp_sb = sm.tile([128, NCORES, B], f32)
            for jn in range(NCORES):
                hp_ps = ps.tile([128, B], f32, tag="hp")
                for mi in range(MCH):
                    nc.tensor.matmul(hp_ps[:],
                                     w2T[:, mi, jn * 128:(jn + 1) * 128],
                                     g1_sb[:, mi, :],
                                     start=(mi == 0), stop=(mi == MCH - 1))
                nc.scalar.copy(hp_sb[:, jn, :], hp_ps[:])
            nc.sync.dma_start(
                HPd[:].rearrange("(jn p) b -> p jn b", p=128), hp_sb[:]
            )

    nc.finalize()
    _built["nc"] = nc
    return nc


def kernel(**inputs):
    from concourse.bass_utils import run_bass_kernel_spmd

    global last_results

    Q = np.asarray(inputs["Q"], dtype=np.float32)
    K = np.asarray(inputs["K"], dtype=np.float32)
    V = np.asarray(inputs["V"], dtype=np.float32)
    mu_w = np.asarray(inputs["mu_w"], dtype=np.float32)
    mu_b = np.asarray(inputs["mu_b"], dtype=np.float32)
    sigma_w = np.asarray(inputs["sigma_w"], dtype=np.float32)
    sigma_b = np.asarray(inputs["sigma_b"], dtype=np.float32)
    ffn_w1 = np.asarray(inputs["ffn_w1"], dtype=np.float32)
    ffn_b1 = np.asarray(inputs["ffn_b1"], dtype=np.float32)
    ffn_w2 = np.asarray(inputs["ffn_w2"], dtype=np.float32)
    ffn_b2 = np.asarray(inputs["ffn_b2"], dtype=np.float32)
    ln_ff_g = np.asarray(inputs["ln_ff_g"], dtype=np.float32)
    ln_ff_b = np.asarray(inputs["ln_ff_b"], dtype=np.float32)
    ln_q_g = np.asarray(inputs["ln_q_g"], dtype=np.float32)
    ln_q_b = np.asarray(inputs["ln_q_b"], dtype=np.float32)

    # ---- Host-side exact folds of LN affine params into next matmuls ----
    # q = t*g + b  =>  q @ W.T + c = t @ (W*g).T + (c + W @ b)
    mu_wf = mu_w * ln_q_g[None, :]
    mu_bf = mu_b + mu_w @ ln_q_b
    sig_wf = sigma_w * ln_q_g[None, :]
    sig_bf = sigma_b + sigma_w @ ln_q_b
    w1f = ffn_w1 * ln_ff_g[None, :]
    b1f = ffn_b1 + ffn_w1 @ ln_ff_b

    QT = np.ascontiguousarray(Q.T)                    # (N, B)
    muwT = np.ascontiguousarray(mu_wf.T).astype(np.float16)   # (N, N)
    sigwT = np.ascontiguousarray(sig_wf.T).astype(np.float16)
    w1T = np.ascontiguousarray(w1f.T).astype(np.float16)      # (N, M)
    w2T = np.ascontiguousarray(ffn_w2.T).astype(np.float16)   # (M, N)
    ident = np.eye(128, dtype=np.float32)
    identh = np.eye(128, dtype=np.float16)
    K16 = K.astype(np.float16)
    V16 = V.astype(np.float16)

    nc = _build_module()

    in_maps = []
    for c in range(NCORES):
        jsl = slice(c * NSL, (c + 1) * NSL)
        msl = slice(c * MSL, (c + 1) * MSL)
        in_maps.append({
            "Ks": np.ascontiguousarray(K16[:, jsl, :].transpose(1, 0, 2)),
            "Vs": np.ascontiguousarray(V16[:, jsl, :].transpose(1, 0, 2)),
            "Qf": Q,
            "QTs": np.ascontiguousarray(QT[jsl, :]),
            "MWT": np.ascontiguousarray(muwT[:, jsl]),
            "SWT": np.ascontiguousarray(sigwT[:, jsl]),
            "MB": np.ascontiguousarray(mu_bf[jsl]).reshape(NSL, 1),
            "SB2": np.ascontiguousarray(
                np.sqrt(2.0).astype(np.float32) * sig_bf[jsl]).reshape(NSL, 1),
            "W1T": np.ascontiguousarray(w1T[:, msl]),
            "B1": np.ascontiguousarray(b1f[msl].reshape(MCH, 128).T),
            "W2T": np.ascontiguousarray(w2T[msl, :]),
            "IDT": ident,
            "IDH": identh,
        })

    trace = os.environ.get("BASS_KERNEL_TRACE", "0") == "1"
    res = run_bass_kernel_spmd(
        nc, in_maps, core_ids=list(range(NCORES)), trace=trace
    )
    last_results = res

    x = np.concatenate([res.results[c]["XT"] for c in range(NCORES)], axis=0).T
    h = np.zeros((N, B), dtype=np.float32)
    for c in range(NCORES):
        h += res.results[c]["HP"]
    out = x + h.T + ffn_b2[None, :]
    return out.astype(np.float32)
